# revision 1
# baseline (speedup 1.0000x reference)
"""Trainium2 Bass kernel for nn_KernelEncoderLayer (gnn_message_passing).

Math (per graph b of 4, N=1024 points, K=9 kernel offsets, C=32 channels):
  y[i,c] = leaky( sum_{n,k} exp(-|pi - pn - kk|^2/2) * (w @ conv_w[k])[n,c] )
  out = BN(y)+w -> MLP(32->128->32, leaky, BN) residual.

Factorization used (k-independent Gaussian Gram matrix):
  exp(-|pi - pn - kk|^2/2) = G[n,i] * A[i,k] * B[n,k]
  G[n,i] = exp(pn.pi - |pn|^2/2 - |pi|^2/2)       (one NxN exp per graph, not NxNxK)
  A[i,k] = exp(pi.kk),  B[n,k] = exp(-pn.kk - |kk|^2/2)   (host-precomputed, O(N*K))
So y[i,c] = sum_k A[i,k] * (G.T @ (B[:,k] * cw[:,k,:]))[i,c].

Sharding: 8 cores = 4 graphs x 2 halves of the 1024 output rows. Each core
computes its [512, 32] conv-output slice; one AllGather ships slices + BN
partial sums; every core then redundantly computes the cheap back half
(BN + residual + MLP + BN) on the full batch; host takes core 0's output.

Self-contained: hardcodes B=4, N=1024, K=9, C=32, CM=128, 8 cores.
"""

import numpy as np

import concourse.bass as bass
import concourse.bacc as bacc
import concourse.mybir as mybir
import concourse.tile as tile
from concourse import masks
from concourse.bass_utils import run_bass_kernel_spmd

F32 = mybir.dt.float32
AF = mybir.ActivationFunctionType
ALU = mybir.AluOpType
AX = mybir.AxisListType

NB, N, K, C, CM = 4, 1024, 9, 32, 128
NCORES = 8
EPS = 1e-5
SLOPE = 0.01


def _build_module():
    nc = bacc.Bacc("TRN2", target_bir_lowering=False, debug=False,
                   num_devices=NCORES)

    def din(name, shape):
        return nc.dram_tensor(name, list(shape), F32, kind="ExternalInput").ap()

    pn3_d = din("pn3", (3, N))           # [pnx; pny; 1] for this core's graph
    pi3_d = din("pi3", (3, 512))         # [pix; piy; -|pi|^2/2] for slice
    negsqn_d = din("negsqn", (128, 8))   # -|pn|^2/2 per n-chunk column
    wT_d = din("wT", (C, N))             # graph weights transposed
    cwall_d = din("cwall", (C, K * C))   # conv_w as [d, k*32+c]
    bcol_d = din("Bcol", (128, 72))      # B[n,k] at [p, nt*9+k]
    acol_d = din("Acol", (128, 36))      # A[i,k] at [p, t*9+k]
    wTo_d = din("wTown", (C, 512))       # weights.T slice for own rows
    w1_d = din("w1", (C, CM))
    w2_d = din("w2", (CM, C))
    b1c_d = din("b1c", (CM, 1))
    g1c_d = din("g1c", (CM, 1))
    be1c_d = din("be1c", (CM, 1))
    g0_d = din("g0", (C, 1))
    be0_d = din("be0", (C, 1))
    b2c_d = din("b2c", (C, 1))

    out_d = nc.dram_tensor("out", [C, 512], F32, kind="ExternalOutput").ap()

    NT = NB * N  # 4096

    with tile.TileContext(nc) as tc:
        with (
            tc.tile_pool(name="const", bufs=1) as pc,
            tc.tile_pool(name="big", bufs=1) as pb,
            tc.tile_pool(name="work", bufs=3) as pw,
            tc.tile_pool(name="psA", bufs=4, space="PSUM") as psA,
            tc.tile_pool(name="psB", bufs=2, space="PSUM") as psB,
            tc.tile_pool(name="psC", bufs=2, space="PSUM") as psC,
            tc.tile_pool(name="dram", bufs=1, space="DRAM") as pd,
        ):
            # ---- load inputs ----
            def load(name, ap, shape, pool=pc):
                t = pool.tile(list(shape), F32, tag=name, name=name)
                nc.sync.dma_start(out=t[:], in_=ap)
                return t

            pn3 = load("pn3", pn3_d, (3, N))
            pi3 = load("pi3", pi3_d, (3, 512))
            negsqn = load("negsqn", negsqn_d, (128, 8))
            wT = load("wT", wT_d, (C, N))
            cwall = load("cwall", cwall_d, (C, K * C))
            bcol = load("Bcol", bcol_d, (128, 72))
            acol = load("Acol", acol_d, (128, 36))
            wTo = load("wTown", wTo_d, (C, 512))
            w1 = load("w1", w1_d, (C, CM))
            w2 = load("w2", w2_d, (CM, C))
            b1c = load("b1c", b1c_d, (CM, 1))
            g1c = load("g1c", g1c_d, (CM, 1))
            be1c = load("be1c", be1c_d, (CM, 1))
            g0 = load("g0", g0_d, (C, 1))
            be0 = load("be0", be0_d, (C, 1))
            b2c = load("b2c", b2c_d, (C, 1))

            ident = pc.tile([128, 128], F32, tag="ident")
            masks.make_identity(nc, ident[:])

            # ---- phase B: cw'[n, (k,c)] = B[n,k] * (w @ conv_w[k]) ----
            cw_sb = [pb.tile([128, K * C], F32, tag=f"cw{nt}", name=f"cw{nt}") for nt in range(8)]
            for nt in range(8):
                ps = psA.tile([128, K * C], F32, tag="mm288")
                nc.tensor.matmul(ps[:], lhsT=wT[:, nt * 128:(nt + 1) * 128],
                                 rhs=cwall[:], start=True, stop=True)
                for k in range(K):
                    nc.vector.tensor_scalar(
                        cw_sb[nt][:, k * C:(k + 1) * C],
                        ps[:, k * C:(k + 1) * C],
                        bcol[:, nt * K + k:nt * K + k + 1], None, op0=ALU.mult)

            # ---- phase C: G[n, i] = exp(pn.pi - |pn|^2/2 - |pi|^2/2) ----
            g_sb = [pb.tile([128, 512], F32, tag=f"g{j}", name=f"g{j}") for j in range(8)]
            for j in range(8):
                ps = psB.tile([128, 512], F32, tag="mm512")
                nc.tensor.matmul(ps[:], lhsT=pn3[:, j * 128:(j + 1) * 128],
                                 rhs=pi3[:], start=True, stop=True)
                nc.scalar.activation(g_sb[j][:], ps[:], AF.Exp,
                                     bias=negsqn[:, j:j + 1], scale=1.0)

            # ---- phase D: main contraction + combine + leaky + transpose ----
            ag_sb = pb.tile([C, 512], F32, tag="ag")  # local yT slice
            ysum_p = pc.tile([C, 4], F32, tag="ysum")
            ysq_p = pc.tile([C, 4], F32, tag="ysq")
            pys = [psA.tile([128, K * C], F32, tag="mm288", name=f"py{t}")
                   for t in range(4)]
            for j in range(8):
                for t in range(4):
                    nc.tensor.matmul(pys[t][:],
                                     lhsT=g_sb[j][:, t * 128:(t + 1) * 128],
                                     rhs=cw_sb[j][:], start=(j == 0), stop=(j == 7))
            for t in range(4):
                py = pys[t]
                y_t = pw.tile([128, C], F32, tag="yt")
                nc.vector.tensor_scalar(
                    y_t[:], py[:, 0:C], acol[:, t * K:t * K + 1], None, op0=ALU.mult)
                for k in range(1, K):
                    nc.vector.scalar_tensor_tensor(
                        y_t[:], py[:, k * C:(k + 1) * C],
                        acol[:, t * K + k:t * K + k + 1], y_t[:],
                        op0=ALU.mult, op1=ALU.add)
                y_l = pw.tile([128, C], F32, tag="yl")
                nc.scalar.activation(y_l[:], y_t[:], AF.Lrelu, alpha=SLOPE)
                ptr = psC.tile([C, 128], F32, tag="tr")
                nc.tensor.transpose(ptr[:], y_l[:], ident[:])
                nc.vector.tensor_scalar(
                    ag_sb[:, t * 128:(t + 1) * 128], ptr[:], 0.0, 0.0,
                    op0=ALU.add, op1=ALU.add, accum_out=ysum_p[:, t:t + 1])
                sq = pw.tile([C, 128], F32, tag="sq")
                agt = ag_sb[:, t * 128:(t + 1) * 128]
                nc.vector.scalar_tensor_tensor(
                    sq[:], agt, 1.0, agt, op0=ALU.mult, op1=ALU.mult,
                    accum_out=ysq_p[:, t:t + 1])
            stat1 = pc.tile([C, 2], F32, tag="stat1")
            nc.vector.tensor_reduce(stat1[:, 0:1], ysum_p[:], axis=AX.X,
                                    op=ALU.add)
            nc.vector.tensor_reduce(stat1[:, 1:2], ysq_p[:], axis=AX.X,
                                    op=ALU.add)

            # ---- AllGather 1: BN0 partial stats only ----
            ag_in = pd.tile([C, 2], F32, tag="ag_in")
            ag_out = pd.tile([C * NCORES, 2], F32, tag="ag_out")
            nc.sync.dma_start(out=ag_in[:], in_=stat1[:])
            nc.gpsimd.collective_compute(
                "AllGather", ALU.bypass,
                replica_groups=[list(range(NCORES))],
                ins=[ag_in[:].opt()], outs=[ag_out[:].opt()])

            stat = pc.tile([C, 2, 8], F32, tag="stat")
            agv = ag_out[:].rearrange("(j p) s -> p s j", j=8)
            for s in range(2):
                nc.sync.dma_start(
                    out=stat[:, s:s + 1, :].opt(),
                    in_=agv[:, s:s + 1, :].opt())
            tot = pc.tile([C, 2], F32, tag="tot")
            nc.vector.tensor_reduce(tot[:], stat[:], axis=AX.X, op=ALU.add)
            mom = pc.tile([C, 2], F32, tag="mom")
            nc.vector.tensor_scalar_mul(mom[:], tot[:], 1.0 / NT)
            var = pc.tile([C, 1], F32, tag="var")
            nc.vector.tensor_tensor(var[:], mom[:, 0:1], mom[:, 0:1], op=ALU.mult)
            nc.vector.scalar_tensor_tensor(var[:], mom[:, 1:2], EPS, var[:],
                                           op0=ALU.add, op1=ALU.subtract)
            std = pc.tile([C, 1], F32, tag="std")
            nc.scalar.activation(std[:], var[:], AF.Sqrt)
            rstd = pc.tile([C, 1], F32, tag="rstd")
            nc.vector.reciprocal(rstd[:], std[:])
            scale0 = pc.tile([C, 1], F32, tag="scale0")
            nc.vector.tensor_tensor(scale0[:], rstd[:], g0[:], op=ALU.mult)
            nscale0 = pc.tile([C, 1], F32, tag="nscale0")
            nc.vector.tensor_scalar_mul(nscale0[:], scale0[:], -1.0)
            shift0 = pc.tile([C, 1], F32, tag="shift0")
            nc.vector.scalar_tensor_tensor(
                shift0[:], mom[:, 0:1], nscale0[:], be0[:],
                op0=ALU.mult, op1=ALU.add)

            # ---- BN0 apply + residual on own slice only ----
            yres = pb.tile([C, 512], F32, tag="yres")
            tmp = pw.tile([C, 512], F32, tag="bn0")
            nc.vector.tensor_scalar(tmp[:], ag_sb[:], scale0[:],
                                    shift0[:], op0=ALU.mult, op1=ALU.add)
            nc.vector.tensor_tensor(yres[:], tmp[:], wTo[:], op=ALU.add)

            # ---- MLP1 on own slice; BN1 stats via AllGather 2 ----
            h_sb = pb.tile([CM, 512], F32, tag="h")
            hstat = pc.tile([CM, 2], F32, tag="hstat")
            ph = psB.tile([CM, 512], F32, tag="mm512")
            nc.tensor.matmul(ph[:], lhsT=w1[:], rhs=yres[:],
                             start=True, stop=True)
            nc.scalar.activation(h_sb[:], ph[:], AF.Lrelu,
                                 bias=b1c[:], scale=1.0, alpha=SLOPE,
                                 accum_out=hstat[:, 0:1])
            sq2 = pw.tile([CM, 512], F32, tag="sq2")
            nc.vector.scalar_tensor_tensor(
                sq2[:], h_sb[:], 1.0, h_sb[:],
                op0=ALU.mult, op1=ALU.mult, accum_out=hstat[:, 1:2])

            ag2_in = pd.tile([CM, 2], F32, tag="ag2_in")
            ag2_out = pd.tile([CM * NCORES, 2], F32, tag="ag2_out")
            nc.sync.dma_start(out=ag2_in[:], in_=hstat[:])
            nc.gpsimd.collective_compute(
                "AllGather", ALU.bypass,
                replica_groups=[list(range(NCORES))],
                ins=[ag2_in[:].opt()], outs=[ag2_out[:].opt()])
            stat2 = pc.tile([CM, 2, 8], F32, tag="stat2")
            ag2v = ag2_out[:].rearrange("(j p) s -> p s j", j=8)
            for s in range(2):
                nc.sync.dma_start(
                    out=stat2[:, s:s + 1, :].opt(),
                    in_=ag2v[:, s:s + 1, :].opt())
            tot1 = pc.tile([CM, 2], F32, tag="tot1")
            nc.vector.tensor_reduce(tot1[:], stat2[:], axis=AX.X, op=ALU.add)
            mom1 = pc.tile([CM, 2], F32, tag="mom1")
            nc.vector.tensor_scalar_mul(mom1[:], tot1[:], 1.0 / NT)
            var1 = pc.tile([CM, 1], F32, tag="var1")
            nc.vector.tensor_tensor(var1[:], mom1[:, 0:1], mom1[:, 0:1], op=ALU.mult)
            nc.vector.scalar_tensor_tensor(var1[:], mom1[:, 1:2], EPS, var1[:],
                                           op0=ALU.add, op1=ALU.subtract)
            std1 = pc.tile([CM, 1], F32, tag="std1")
            nc.scalar.activation(std1[:], var1[:], AF.Sqrt)
            rstd1 = pc.tile([CM, 1], F32, tag="rstd1")
            nc.vector.reciprocal(rstd1[:], std1[:])
            scale1 = pc.tile([CM, 1], F32, tag="scale1")
            nc.vector.tensor_tensor(scale1[:], rstd1[:], g1c[:], op=ALU.mult)
            nscale1 = pc.tile([CM, 1], F32, tag="nscale1")
            nc.vector.tensor_scalar_mul(nscale1[:], scale1[:], -1.0)
            shift1 = pc.tile([CM, 1], F32, tag="shift1")
            nc.vector.scalar_tensor_tensor(
                shift1[:], mom1[:, 0:1], nscale1[:], be1c[:],
                op0=ALU.mult, op1=ALU.add)

            # ---- BN1 apply + MLP2 + final residual (own slice) ----
            out_sb = pb.tile([C, 512], F32, tag="outsb")
            hb = pw.tile([CM, 512], F32, tag="hbn")
            nc.vector.tensor_scalar(hb[:], h_sb[:], scale1[:],
                                    shift1[:], op0=ALU.mult, op1=ALU.add)
            pdl = psB.tile([C, 512], F32, tag="mm512")
            nc.tensor.matmul(pdl[:], lhsT=w2[:], rhs=hb[:],
                             start=True, stop=True)
            nc.vector.scalar_tensor_tensor(
                out_sb[:], pdl[:], b2c[:], yres[:],
                op0=ALU.add, op1=ALU.add)
            nc.sync.dma_start(out=out_d, in_=out_sb[:])

    nc.compile()
    return nc


_NC_CACHE = {}


def _get_module():
    if "nc" not in _NC_CACHE:
        _NC_CACHE["nc"] = _build_module()
    return _NC_CACHE["nc"]


def _host_prep(inputs):
    pos = np.asarray(inputs["positions"], np.float32)
    w = np.asarray(inputs["weights"], np.float32)
    kp = np.asarray(inputs["kernel_pos"], np.float32)
    cw = np.asarray(inputs["conv_w"], np.float32)
    posb = pos.reshape(NB, N, 2)
    wb = w.reshape(NB, N, C)
    kk2 = 0.5 * (kp ** 2).sum(1)                       # [9]
    cwall = np.ascontiguousarray(cw.transpose(1, 0, 2).reshape(C, K * C))
    wTfull = np.ascontiguousarray(w.T)                 # [32, 4096]
    w1 = np.asarray(inputs["w1"], np.float32)
    w2 = np.asarray(inputs["w2"], np.float32)
    shared = dict(
        cwall=cwall, w1=w1, w2=w2,
        b1c=np.asarray(inputs["b1"], np.float32).reshape(CM, 1),
        g1c=np.asarray(inputs["bn1_gamma"], np.float32).reshape(CM, 1),
        be1c=np.asarray(inputs["bn1_beta"], np.float32).reshape(CM, 1),
        g0=np.asarray(inputs["bn_gamma"], np.float32).reshape(C, 1),
        be0=np.asarray(inputs["bn_beta"], np.float32).reshape(C, 1),
        b2c=np.asarray(inputs["b2"], np.float32).reshape(C, 1),
    )
    in_maps = []
    for j in range(NCORES):
        b, off = j // 2, (j % 2) * 512
        p = posb[b]
        pi = p[off:off + 512]
        pn3 = np.ascontiguousarray(
            np.stack([p[:, 0], p[:, 1], np.ones(N, np.float32)]))
        pi3 = np.ascontiguousarray(
            np.stack([pi[:, 0], pi[:, 1], -0.5 * (pi ** 2).sum(1)]))
        negsqn = np.ascontiguousarray(
            (-0.5 * (p ** 2).sum(1)).reshape(8, 128).T)
        dotn = (p @ kp.T).astype(np.float32)            # [1024, 9]
        Bmat = np.exp(-dotn - kk2[None, :]).astype(np.float32)
        bcol = np.ascontiguousarray(
            Bmat.reshape(8, 128, K).transpose(1, 0, 2).reshape(128, 72))
        Amat = np.exp((pi @ kp.T).astype(np.float32)).astype(np.float32)
        acol = np.ascontiguousarray(
            Amat.reshape(4, 128, K).transpose(1, 0, 2).reshape(128, 36))
        wT = np.ascontiguousarray(wb[b].T)
        m = dict(pn3=pn3, pi3=pi3, negsqn=negsqn, wT=wT,
                 Bcol=bcol, Acol=acol,
                 wTown=np.ascontiguousarray(wTfull[:, j * 512:(j + 1) * 512]))
        m.update(shared)
        in_maps.append(m)
    return in_maps


def _run(inputs, trace=False):
    nc = _get_module()
    in_maps = _host_prep(inputs)
    res = run_bass_kernel_spmd(nc, in_maps, core_ids=list(range(NCORES)),
                               trace=trace)
    out = np.concatenate([np.asarray(res.results[j]["out"])
                          for j in range(NCORES)], axis=1)   # [32, 4096]
    return np.ascontiguousarray(out.T), res


def kernel(**inputs):
    out, _ = _run(inputs, trace=False)
    return out

